# revision 14
# baseline (speedup 1.0000x reference)
"""Cached multi-head attention on 8 TRN2 NeuronCores.

Sharding: core c = 2*b + g handles batch b (of 4) and head-group g (of 2,
8 heads each) -- data parallel on batch x tensor parallel on heads.
Column-parallel Wq/Wk/Wv, row-parallel Wo; the Wo all-reduce (sum of the
two head-group partials per batch) is done on host during the unshard,
along with the bo bias add.

Device layout (per core), all matmuls bf16 in / f32 psum out:
  xT = x.T in HBM (host pre-transposed). Projections:
    qT[d,t] = sum_c WqT[c,d] xqT[c,t]  (+bq)   -> SBUF pair tiles [128, T]
    kT likewise; v[s,d] = sum_c xvT[c,s] WvT[c,d] (+bv via DVE add of a
    partition-broadcast bias tile)
  Attention per head-pair (2 heads row-packed in the 128-partition dim):
    ST[s,t] = kT.T @ qT   (K=64 row-tiled, both heads concurrent)
    P = exp(ST/8)         (ScalarE, free scale; no max-subtract needed --
                           scores are O(1) by construction)
    oT_aug = [V|1].T @ P  (K=128, M=65; row 64 = softmax denominators)
    o = oT * (1/denom)    (DVE mult with gpsimd-broadcast reciprocal)
  Out-projection: out[t,e] = sum_d oT[d,t] WoT[d,e], accumulated over the
  4 pair-chunks of d, written bf16 (host sums the two partials in f32).

Schedule: the per-block chain score->exp->PV is ScalarE-bound (exp ~1.2us
vs ~0.8us of PE per block), so PE "filler" work (projections for the next
chunk, deferred out-projections) is drained INSIDE each attention unit
between blocks.  All out-projection for chunks 0..2 is deferred into the
final (most exp-bound) chunk.  Startup uses per-128-row chunked DMAs for
wv/xv so the first matmul fires as soon as the first 256KB land.

Causal masks get a fast path: blocks above the diagonal are skipped,
diagonal blocks use shortened matmuls + one 3D-pattern gpsimd
affine_select zeroing both heads at once.  Arbitrary masks fall back to
per-block skip/plain/mixed classification with host-shipped
multiplicative mask tiles.
"""

import math
import ml_dtypes
import numpy as np

import concourse.bass as bass
import concourse.mybir as mybir
import concourse.tile as tile
from concourse import bacc
from concourse.bass_utils import run_bass_kernel_spmd

F32 = mybir.dt.float32
BF16 = mybir.dt.bfloat16
AF = mybir.ActivationFunctionType
ts = bass.ts

B, T, D, H = 4, 2048, 1024, 16
HD = D // H          # 64
NCORE = 8
DG = D // 2          # 512 dims per core (8 heads)
NPAIR = 4            # head pairs per core
SB = 128             # s-block size
TC = 512             # attention t-chunk
NTC = T // TC        # 4
NSB = T // SB        # 16
PC = 512             # projection t-chunk (x streaming granularity)
NPC = T // PC        # 4
CCH = D // 128       # 8 contraction chunks

_cache = {}
last_result = {}


def _classify_blocks(mask):
    """Per (s_blk, t_chunk) classification, unioned across batches (SPMD).

    Returns (mode, cls, mixed_list) where cls[s][i] in {0 skip, 1 plain,
    2 mixed} and mixed_list orders the mixed blocks.
    """
    causal = np.triu(np.ones((T, T), dtype=bool), k=1)
    if all(np.array_equal(mask[b], causal) for b in range(B)):
        return "causal", None, None
    cls = np.zeros((NSB, NTC), dtype=np.int64)
    for s in range(NSB):
        for i in range(NTC):
            per_b_all = [mask[b, i * TC:(i + 1) * TC, s * SB:(s + 1) * SB].all()
                         for b in range(B)]
            per_b_any = [mask[b, i * TC:(i + 1) * TC, s * SB:(s + 1) * SB].any()
                         for b in range(B)]
            if all(per_b_all):
                cls[s, i] = 0
            elif not any(per_b_any):
                cls[s, i] = 1
            else:
                cls[s, i] = 2
    mixed = [(s, i) for s in range(NSB) for i in range(NTC) if cls[s, i] == 2]
    return "general", cls, mixed


def _build(mode, cls, n_mixed):
    nc = bacc.Bacc("TRN2", target_bir_lowering=False, debug=False,
                   num_devices=NCORE)
    d = {}
    for nm in ("xq", "xk", "xv"):
        d[nm] = nc.dram_tensor(nm, [D, T], BF16, kind="ExternalInput").ap()
    d["wv"] = nc.dram_tensor("wv", [D, DG], BF16, kind="ExternalInput").ap()
    # wq/wk are host-swizzled pair-major: [pair, r, c, e] so one DMA pulls
    # a single pair's weights with 2KB-contiguous per-partition rows
    for nm in ("wq", "wk"):
        d[nm] = nc.dram_tensor(nm, [NPAIR * 128 * CCH, 128], BF16,
                               kind="ExternalInput").ap()
    d["wo"] = nc.dram_tensor("wo", [DG, D], BF16, kind="ExternalInput").ap()
    d["bq"] = nc.dram_tensor("bq", [128, NPAIR], F32, kind="ExternalInput").ap()
    d["bk"] = nc.dram_tensor("bk", [128, NPAIR], F32, kind="ExternalInput").ap()
    d["bv"] = nc.dram_tensor("bv", [128, DG], BF16, kind="ExternalInput").ap()
    if n_mixed:
        d["mmask"] = nc.dram_tensor("mmask", [n_mixed, SB, TC], BF16,
                                    kind="ExternalInput").ap()
    out_d = nc.dram_tensor("out", [T, D], BF16, kind="ExternalOutput").ap()

    with tile.TileContext(nc) as tc:
        with (
            tc.tile_pool(name="persist", bufs=1) as pp,
            tc.tile_pool(name="stream", bufs=2) as sp,
            tc.tile_pool(name="small", bufs=2) as mp,
            tc.tile_pool(name="psum", bufs=2, space="PSUM") as psp,
        ):
            HV = HD + 1  # 65: V columns + ones column per head

            # ---- persistent SBUF tiles ----------------------------------
            wv_sb = pp.tile([128, CCH * DG], BF16, tag="wv")
            w_sb = {"wv": wv_sb}
            for nm in ("wq", "wk"):
                w_sb[nm] = pp.tile([128, CCH * DG], BF16, tag=nm, name=nm + "_sb")
            wo_sb = pp.tile([128, NPAIR * D], BF16, tag="wo")
            bq_sb = pp.tile([128, NPAIR], F32, tag="bq")
            bk_sb = pp.tile([128, NPAIR], F32, tag="bk")
            bvb_sb = pp.tile([128, DG], BF16, tag="bvb")
            v_sb = [pp.tile([128, 8 * HV], BF16, tag=f"v{s}", name=f"v{s}")
                    for s in range(NSB)]
            qT = [[pp.tile([128, TC], BF16, tag=f"qT{p}_{i}", name=f"qT{p}_{i}")
                   for i in range(NTC)] for p in range(NPAIR)]
            kT = [[pp.tile([128, TC], BF16, tag=f"kT{p}_{i}", name=f"kT{p}_{i}")
                   for i in range(NTC)] for p in range(NPAIR)]
            oT = [[pp.tile([128, TC], BF16, tag=f"oT{p}_{i}", name=f"oT{p}_{i}")
                   for i in range(NTC)] for p in range(NPAIR)]

            # ---- input prefetch -----------------------------------------
            # x chunk tiles keyed by tau; all big loads via gpsimd SW-DGE
            # (16 queues).  Chunk 0 + wv are split per 128-row c-chunk so
            # the first V matmul fires after ~256KB instead of ~7MB.
            x_tiles = {}

            def x_dram(nm):
                return d[nm].rearrange("(c p) t -> p c t", p=128)

            def prefetch_x(tau, fine=False):
                xs = []
                for nm in ("xv", "xq", "xk"):
                    x = sp.tile([128, CCH * PC], BF16, tag="x", bufs=6,
                                name=f"{nm}_t{tau}")
                    xs.append(x)
                x_tiles[tau] = xs
                if not fine:
                    for x, nm in zip(xs, ("xv", "xq", "xk")):
                        nc.gpsimd.dma_start(
                            out=x[:].rearrange("p (c t) -> p c t", t=PC),
                            in_=x_dram(nm)[:, :, ts(tau, PC)])

            def fine_start_loads():
                # interleave wv-chunk / xv-chunk DMAs so (wv_c0, xv_c0)
                # land first and matmuls can start immediately; the
                # qk-path loads go out in halves on the two HWDGE queues
                # (scalar/sync) + gpsimd SW-DGE concurrently.
                prefetch_x(0, fine=True)
                xv, xq, xk = x_tiles[0]

                def load_w_pair(nm, p, eng):
                    eng.dma_start(
                        out=w_sb[nm][:].rearrange(
                            "p (c e) -> p c e", e=DG)[:, :, p * 128:(p + 1) * 128],
                        in_=d[nm].rearrange(
                            "(q r c) e -> r q c e", r=128, c=CCH)[:, p, :, :])

                # gpsimd dma_start ISSUE costs ~1.1us each (SW-DGE desc
                # gen), so the startup prefix keeps gpsimd to 8 coarse
                # issues and pushes the qk-path issues onto the scalar/
                # sync HWDGE engines (~0.6us, run concurrently).  The ones
                # memsets go to the idle vector queue and the V bias comes
                # pre-broadcast from the host.
                for s in range(NSB):
                    ones_cols = v_sb[s][:].rearrange(
                        "p (h c) -> p h c", c=HV)[:, :, HD:HV]
                    nc.vector.memset(ones_cols, 1.0)
                load_w_pair("wq", 0, nc.scalar)
                load_w_pair("wk", 0, nc.sync)
                for c2 in range(CCH // 2):
                    nc.gpsimd.dma_start(
                        out=wv_sb[:, ts(c2, 2 * DG)],
                        in_=d["wv"].rearrange(
                            "(c p) e -> p c e", p=128)[:, 2 * c2:2 * c2 + 2, :])
                    nc.gpsimd.dma_start(
                        out=xv[:].rearrange(
                            "p (c t) -> p c t", t=PC)[:, 2 * c2:2 * c2 + 2, :],
                        in_=x_dram("xv")[:, 2 * c2:2 * c2 + 2, ts(0, PC)])
                for h in range(2):
                    nc.scalar.dma_start(
                        out=xq[:].rearrange("p (c t) -> p c t", t=PC)[:, 4 * h:4 * h + 4, :],
                        in_=x_dram("xq")[:, 4 * h:4 * h + 4, ts(0, PC)])
                    nc.sync.dma_start(
                        out=xk[:].rearrange("p (c t) -> p c t", t=PC)[:, 4 * h:4 * h + 4, :],
                        in_=x_dram("xk")[:, 4 * h:4 * h + 4, ts(0, PC)])
                nc.sync.dma_start(out=bvb_sb[:], in_=d["bv"][:])
                for p_, eng in ((1, nc.scalar), (2, nc.sync), (3, nc.scalar)):
                    load_w_pair("wq", p_, eng)
                    load_w_pair("wk", p_, eng)
                nc.sync.dma_start(out=bq_sb[:], in_=d["bq"][:])
                nc.sync.dma_start(out=bk_sb[:], in_=d["bk"][:])
                nc.gpsimd.dma_start(
                    out=wo_sb[:].rearrange("p (c e) -> p c e", e=D),
                    in_=d["wo"].rearrange("(c p) e -> p c e", p=128))

            # ---- V projection (per 128-token group) ---------------------
            def emit_v_group(tau, u):
                x = x_tiles[tau][0]
                sigma = tau * (PC // SB) + u
                ps = psp.tile([128, TC], F32, tag="b512", bufs=2)
                for c in range(CCH):
                    nc.tensor.matmul(
                        ps[:],
                        x[:, c * PC + u * SB:c * PC + (u + 1) * SB],
                        wv_sb[:, ts(c, DG)],
                        start=(c == 0), stop=(c == CCH - 1))
                vdst = v_sb[sigma][:].rearrange("p (h c) -> p h c", c=HV)[:, :, 0:HD]
                vsrc = ps[:].rearrange("p (h c) -> p h c", c=HD)
                bvv = bvb_sb[:].rearrange("p (h c) -> p h c", c=HD)
                nc.vector.tensor_add(vdst, vsrc, bvv)

            # ---- Q/K projections (per (pair, q-or-k) psum group) --------
            def emit_qk_one(tau, p, nm):
                xx = x_tiles[tau][1 if nm == "q" else 2]
                dst, bias = (qT, bq_sb) if nm == "q" else (kT, bk_sb)
                ps = psp.tile([128, TC], F32, tag="b512", bufs=2)
                for c in range(CCH):
                    nc.tensor.matmul(
                        ps[:],
                        w_sb["w" + nm][:, c * DG + p * 128:c * DG + (p + 1) * 128],
                        xx[:, ts(c, PC)],
                        start=(c == 0), stop=(c == CCH - 1))
                nc.vector.tensor_scalar(
                    out=dst[p][tau][:], in0=ps[:],
                    scalar1=bias[:, p:p + 1], scalar2=None,
                    op0=mybir.AluOpType.add)

            scale = 1.0 / math.sqrt(HD)

            def build_unit(i, p):
                """Returns (st_fns, pv_fns, epi_fn) for attention unit (i,p)."""
                if mode == "causal":
                    blocks = []
                    for s_blk in range(4 * i + 4):
                        j = s_blk - 4 * i
                        if j < 0:
                            blocks.append((s_blk, i * TC, TC, False))
                        else:
                            s0 = SB * s_blk
                            toff = s0 if j < 3 else s0 - SB
                            blocks.append((s_blk, toff, TC * (i + 1) - toff, True))
                else:
                    blocks = [(s_blk, i * TC, TC, False)
                              for s_blk in range(NSB) if cls[s_blk, i] != 0]
                state = {"p2": {}, "ot": None}

                def make_st(bi):
                    s_blk, toff, n, diag = blocks[bi]

                    def fn():
                        s0 = SB * s_blk
                        sc, lo = s_blk // 4, SB * (s_blk % 4)
                        tl = toff - i * TC
                        st2 = psp.tile([128, 2 * TC], F32, tag="stAB", bufs=2,
                                       name="st2")
                        nc.tensor.matmul(
                            st2[:, 0:n], kT[p][sc][0:HD, lo:lo + SB],
                            qT[p][i][0:HD, tl:tl + n],
                            start=True, stop=True, tile_position=(0, 0))
                        nc.tensor.matmul(
                            st2[:, TC:TC + n], kT[p][sc][HD:128, lo:lo + SB],
                            qT[p][i][HD:128, tl:tl + n],
                            start=True, stop=True, tile_position=(64, 0))
                        p2 = sp.tile([128, 2 * TC], BF16, tag="pAB", bufs=6,
                                     name="p2")
                        if n == TC:
                            nc.scalar.activation(p2[:], st2[:], AF.Exp, scale=scale)
                        else:
                            st3 = st2[:].rearrange("p (b c) -> p b c", b=2)[:, :, 0:n]
                            p3 = p2[:].rearrange("p (b c) -> p b c", b=2)[:, :, 0:n]
                            nc.scalar.activation(p3, st3, AF.Exp, scale=scale)
                        if mode == "causal" and diag:
                            w_ = s0 + SB - toff
                            p3w = p2[:].rearrange("p (b c) -> p b c", b=2)[:, :, 0:w_]
                            nc.gpsimd.affine_select(
                                out=p3w, in_=p3w,
                                compare_op=mybir.AluOpType.is_ge,
                                fill=0.0, base=toff - s0,
                                pattern=[[0, 2], [1, w_]], channel_multiplier=-1)
                        elif mode == "general" and cls[s_blk, i] == 2:
                            mmt = sp.tile([SB, TC], BF16, tag="mmask", name="mmt")
                            nc.sync.dma_start(out=mmt[:],
                                              in_=d["mmask"][mixed_idx[(s_blk, i)]])
                            for off in (0, TC):
                                nc.vector.tensor_mul(p2[:, off:off + n],
                                                     p2[:, off:off + n], mmt[:, 0:n])
                        state["p2"][bi] = p2
                    return fn

                def make_pv(bi):
                    s_blk, toff, n, diag = blocks[bi]

                    def fn():
                        if state["ot"] is None:
                            state["ot"] = (
                                psp.tile([HV, TC], F32, tag="ot", bufs=2, name="otA"),
                                psp.tile([HV, TC], F32, tag="ot", bufs=2, name="otB"))
                        otA, otB = state["ot"]
                        p2 = state["p2"].pop(bi)
                        tl = toff - i * TC
                        vv = v_sb[s_blk][:].rearrange("p (h c) -> p h c", c=HV)
                        first, last = bi == 0, bi == len(blocks) - 1
                        nc.tensor.matmul(otA[:, tl:tl + n], vv[:, 2 * p, :],
                                         p2[:, 0:n], start=first, stop=last)
                        nc.tensor.matmul(otB[:, tl:tl + n], vv[:, 2 * p + 1, :],
                                         p2[:, TC:TC + n], start=first, stop=last)
                    return fn

                def epi():
                    otA, otB = state["ot"]
                    # head B first: its extra SBUF->SBUF hop (stg DMA into
                    # partitions 64..127) is the long pole before the
                    # out-projection can consume oT.
                    for hh, ot_ps in ((1, otB), (0, otA)):
                        den = mp.tile([HV, TC], F32, tag="den", name="den")
                        nc.vector.tensor_copy(den[HD:HV, :], ot_ps[HD:HV, :])
                        r1 = mp.tile([1, TC], F32, tag="r1", name="r1")
                        nc.sync.dma_start(out=r1[:], in_=den[HD:HV, :])
                        rbd = mp.tile([HD, TC], F32, tag="rbd", name="rbd")
                        nc.gpsimd.partition_broadcast(rbd[:], r1[:])
                        rb = mp.tile([HD, TC], F32, tag="rb", name="rb")
                        nc.vector.reciprocal_approx_fast(out=rb[:], in_=rbd[:])
                        if hh == 0:
                            nc.vector.tensor_mul(oT[p][i][0:HD, :], ot_ps[0:HD, :],
                                                 rb[:])
                        else:
                            stg = mp.tile([HD, TC], BF16, tag="stg", name="stg")
                            nc.vector.tensor_mul(stg[:], ot_ps[0:HD, :], rb[:])
                            nc.sync.dma_start(out=oT[p][i][HD:128, :], in_=stg[:])

                n = len(blocks)
                return [make_st(b) for b in range(n)], [make_pv(b) for b in range(n)], epi

            # ---- out-projection (per (chunk, tt) piece: both e-halves) --
            def emit_outproj_tt(i, tt):
                ob = sp.tile([128, D], BF16, tag="ob", bufs=3, name="ob")
                for e in range(2):
                    ps = psp.tile([128, TC], F32, tag="b512", bufs=2, name="ops")
                    for p in range(NPAIR):
                        nc.tensor.matmul(
                            ps[:], oT[p][i][:, ts(tt - 4 * i, 128)],
                            wo_sb[:, p * D + e * TC:p * D + (e + 1) * TC],
                            start=(p == 0), stop=(p == NPAIR - 1))
                    nc.vector.tensor_copy(ob[:, ts(e, TC)], ps[:])
                    # per-half DMA on alternating queues: e=0 overlaps the
                    # e=1 matmuls, and the final store drains ~2x faster
                    eng = nc.sync if e == 0 else nc.gpsimd
                    eng.dma_start(out=out_d[ts(tt, 128), ts(e, TC)],
                                  in_=ob[:, ts(e, TC)])

            # ---- schedule ------------------------------------------------
            # LAG-pipelined attention emission (PVs lag STs by LAG blocks,
            # carried across unit boundaries) with a per-chunk "pieces"
            # queue of PE filler drained between blocks inside each unit.
            LAG = 1
            carry = []

            def emit_unit(st_fns, pv_fns, epi, pieces, quota):
                prev = carry[:]
                carry.clear()
                nb = len(st_fns)
                done = 0
                for b in range(nb):
                    st_fns[b]()
                    if b < len(prev):
                        prev[b]()
                    if b >= LAG:
                        pv_fns[b - LAG]()
                    want = min(quota * (b + 1) // nb, len(pieces))
                    while done < want:
                        pieces.pop(0)()
                        done -= -1
                for fn in prev[nb:]:
                    fn()
                carry.extend(pv_fns[max(nb - LAG, 0):])
                carry.append(epi)

            fine_start_loads()
            for u in range(PC // SB):
                emit_v_group(0, u)
            emit_qk_one(0, 0, "q")
            emit_qk_one(0, 0, "k")

            for i in range(NTC):
                pieces = []
                t2 = i + 1
                if t2 < NTC:
                    pieces.append(lambda t2=t2: prefetch_x(t2))
                    for u in range(PC // SB):
                        pieces.append(lambda t2=t2, u=u: emit_v_group(t2, u))
                    for p_ in range(NPAIR):
                        for nm in ("q", "k"):
                            pieces.append(
                                lambda t2=t2, p_=p_, nm=nm: emit_qk_one(t2, p_, nm))
                else:
                    for j in range(NTC - 1):
                        for tt in range(4 * j, 4 * j + 4):
                            pieces.append(lambda j=j, tt=tt: emit_outproj_tt(j, tt))
                for p in range(NPAIR):
                    if i == 0 and p + 1 < NPAIR:
                        # chunk 0: q/k for pair p+1 must precede unit (0,p+1)
                        emit_qk_one(0, p + 1, "q")
                        emit_qk_one(0, p + 1, "k")
                    st_fns, pv_fns, epi = build_unit(i, p)
                    quota = -(-len(pieces) // (NPAIR - p))
                    emit_unit(st_fns, pv_fns, epi, pieces, quota)
                for fn in pieces:
                    fn()
            # tail: overlap the last unit's epilogue chain with partial
            # out-projection accumulation (pairs 0..2 need no epilogue of
            # the final pair).  tt=12 uses the b512 pool pair, tt=13 the
            # just-freed stAB-sized psum (two 512-col halves).
            *pvs, epi_last = carry
            for fn in pvs:
                fn()
            i3 = NTC - 1
            ps12 = [psp.tile([128, TC], F32, tag="b512", bufs=2, name="ops")
                    for _ in range(2)]
            ps13 = psp.tile([128, 2 * TC], F32, tag="stAB", bufs=2, name="ops13")
            ps14 = psp.tile([128, 2 * TC], F32, tag="stAB", bufs=2, name="ops14")
            part = {(12, 0): ps12[0][:], (12, 1): ps12[1][:],
                    (13, 0): ps13[:, 0:TC], (13, 1): ps13[:, TC:2 * TC],
                    (14, 0): ps14[:, 0:TC], (14, 1): ps14[:, TC:2 * TC]}
            for (tt, e), ps in part.items():
                for p in range(NPAIR - 1):
                    nc.tensor.matmul(
                        ps, oT[p][i3][:, ts(tt - 4 * i3, 128)],
                        wo_sb[:, p * D + e * TC:p * D + (e + 1) * TC],
                        start=(p == 0), stop=False)
            epi_last()
            for tt in (12, 13, 14):
                ob = sp.tile([128, D], BF16, tag="ob", bufs=3, name="ob")
                for e in range(2):
                    ps = part[(tt, e)]
                    nc.tensor.matmul(
                        ps, oT[NPAIR - 1][i3][:, ts(tt - 4 * i3, 128)],
                        wo_sb[:, (NPAIR - 1) * D + e * TC:(NPAIR - 1) * D + (e + 1) * TC],
                        start=False, stop=True)
                    nc.vector.tensor_copy(ob[:, ts(e, TC)], ps)
                    eng = nc.sync if e == 0 else nc.gpsimd
                    eng.dma_start(out=out_d[ts(tt, 128), ts(e, TC)],
                                  in_=ob[:, ts(e, TC)])
            emit_outproj_tt(i3, 15)

    nc.compile()
    return nc


def kernel(**inputs):
    query = np.asarray(inputs["query"], np.float32)
    key = np.asarray(inputs["key"], np.float32)
    value = np.asarray(inputs["value"], np.float32)
    mask = np.asarray(inputs["mask"], bool)
    Wq, bq = np.asarray(inputs["Wq"], np.float32), np.asarray(inputs["bq"], np.float32)
    Wk, bk = np.asarray(inputs["Wk"], np.float32), np.asarray(inputs["bk"], np.float32)
    Wv, bv = np.asarray(inputs["Wv"], np.float32), np.asarray(inputs["bv"], np.float32)
    Wo, bo = np.asarray(inputs["Wo"], np.float32), np.asarray(inputs["bo"], np.float32)

    mode, cls, mixed = _classify_blocks(mask)
    global mixed_idx
    if mode == "general":
        mixed_idx = {blk: n for n, blk in enumerate(mixed)}
        n_mixed = len(mixed)
    else:
        mixed_idx, n_mixed = {}, 0

    key_sig = (mode, tuple(cls.ravel()) if cls is not None else None)
    if key_sig not in _cache:
        _cache[key_sig] = _build(mode, cls, n_mixed)
    nc = _cache[key_sig]

    in_maps = []
    xT = {}
    for b in range(B):
        xT[("xq", b)] = np.ascontiguousarray(query[b].T).astype(ml_dtypes.bfloat16)
        xT[("xk", b)] = np.ascontiguousarray(key[b].T).astype(ml_dtypes.bfloat16)
        xT[("xv", b)] = np.ascontiguousarray(value[b].T).astype(ml_dtypes.bfloat16)
    for core in range(NCORE):
        b, g = core // 2, core % 2
        sl = slice(g * DG, (g + 1) * DG)
        im = {
            "xq": xT[("xq", b)], "xk": xT[("xk", b)], "xv": xT[("xv", b)],
            # pair-major swizzle [pair, r, c, e] flattened to 2D
            "wq": np.ascontiguousarray(
                Wq[sl, :].T.reshape(CCH, 128, NPAIR, 128)
                .transpose(2, 1, 0, 3).reshape(NPAIR * 128 * CCH, 128)
            ).astype(ml_dtypes.bfloat16),
            "wk": np.ascontiguousarray(
                Wk[sl, :].T.reshape(CCH, 128, NPAIR, 128)
                .transpose(2, 1, 0, 3).reshape(NPAIR * 128 * CCH, 128)
            ).astype(ml_dtypes.bfloat16),
            "wv": np.ascontiguousarray(Wv[sl, :].T).astype(ml_dtypes.bfloat16),
            "wo": np.ascontiguousarray(Wo[:, sl].T).astype(ml_dtypes.bfloat16),
            "bq": np.ascontiguousarray(bq[sl].reshape(NPAIR, 128).T),
            "bk": np.ascontiguousarray(bk[sl].reshape(NPAIR, 128).T),
            "bv": np.ascontiguousarray(
                np.broadcast_to(bv[sl][None, :], (128, DG))).astype(ml_dtypes.bfloat16),
        }
        if n_mixed:
            mm = np.empty((n_mixed, SB, TC), ml_dtypes.bfloat16)
            for n, (s_blk, i) in enumerate(mixed):
                blk = mask[b, i * TC:(i + 1) * TC, s_blk * SB:(s_blk + 1) * SB]
                mm[n] = (~blk.T).astype(np.float32)
            im["mmask"] = mm
        in_maps.append(im)

    r = run_bass_kernel_spmd(nc, in_maps, core_ids=list(range(NCORE)))
    last_result["exec_time_ns"] = r.exec_time_ns
    out = np.empty((B, T, D), np.float32)
    for b in range(B):
        out[b] = (r.results[2 * b]["out"].astype(np.float32)
                  + r.results[2 * b + 1]["out"].astype(np.float32))
    out += bo[None, None, :]
    return out


# revision 15
# speedup vs baseline: 1.0463x; 1.0463x over previous
"""Cached multi-head attention on 8 TRN2 NeuronCores.

Sharding: core c = 2*b + g handles batch b (of 4) and head-group g (of 2,
8 heads each) -- data parallel on batch x tensor parallel on heads.
Column-parallel Wq/Wk/Wv, row-parallel Wo; the Wo all-reduce (sum of the
two head-group partials per batch) is done on host during the unshard,
along with the bo bias add.

Device layout (per core), all matmuls bf16 in / f32 psum out:
  xT = x.T in HBM (host pre-transposed). Projections:
    qT[d,t] = sum_c WqT[c,d] xqT[c,t]  (+bq)   -> SBUF pair tiles [128, T]
    kT likewise; v[s,d] = sum_c xvT[c,s] WvT[c,d] (+bv via DVE add of a
    partition-broadcast bias tile)
  Attention per head-pair (2 heads row-packed in the 128-partition dim):
    ST[s,t] = kT.T @ qT   (K=64 row-tiled, both heads concurrent)
    P = exp(ST/8)         (ScalarE, free scale; no max-subtract needed --
                           scores are O(1) by construction)
    oT_aug = [V|1].T @ P  (K=128, M=65; row 64 = softmax denominators)
    o = oT * (1/denom)    (DVE mult with gpsimd-broadcast reciprocal)
  Out-projection: out[t,e] = sum_d oT[d,t] WoT[d,e], accumulated over the
  4 pair-chunks of d, written bf16 (host sums the two partials in f32).

Schedule: the per-block chain score->exp->PV is ScalarE-bound (exp ~1.2us
vs ~0.8us of PE per block), so PE "filler" work (projections for the next
chunk, deferred out-projections) is drained INSIDE each attention unit
between blocks.  All out-projection for chunks 0..2 is deferred into the
final (most exp-bound) chunk.  Startup uses per-128-row chunked DMAs for
wv/xv so the first matmul fires as soon as the first 256KB land.

Causal masks get a fast path: blocks above the diagonal are skipped,
diagonal blocks use shortened matmuls + one 3D-pattern gpsimd
affine_select zeroing both heads at once.  Arbitrary masks fall back to
per-block skip/plain/mixed classification with host-shipped
multiplicative mask tiles.
"""

import math
import ml_dtypes
import numpy as np

import concourse.bass as bass
import concourse.mybir as mybir
import concourse.tile as tile
from concourse import bacc
from concourse.bass_utils import run_bass_kernel_spmd

F32 = mybir.dt.float32
BF16 = mybir.dt.bfloat16
AF = mybir.ActivationFunctionType
ts = bass.ts

B, T, D, H = 4, 2048, 1024, 16
HD = D // H          # 64
NCORE = 8
DG = D // 2          # 512 dims per core (8 heads)
NPAIR = 4            # head pairs per core
SB = 128             # s-block size
TC = 512             # attention t-chunk
NTC = T // TC        # 4
NSB = T // SB        # 16
PC = 512             # projection t-chunk (x streaming granularity)
NPC = T // PC        # 4
CCH = D // 128       # 8 contraction chunks

_cache = {}
last_result = {}


def _classify_blocks(mask):
    """Per (s_blk, t_chunk) classification, unioned across batches (SPMD).

    Returns (mode, cls, mixed_list) where cls[s][i] in {0 skip, 1 plain,
    2 mixed} and mixed_list orders the mixed blocks.
    """
    causal = np.triu(np.ones((T, T), dtype=bool), k=1)
    if all(np.array_equal(mask[b], causal) for b in range(B)):
        return "causal", None, None
    cls = np.zeros((NSB, NTC), dtype=np.int64)
    for s in range(NSB):
        for i in range(NTC):
            per_b_all = [mask[b, i * TC:(i + 1) * TC, s * SB:(s + 1) * SB].all()
                         for b in range(B)]
            per_b_any = [mask[b, i * TC:(i + 1) * TC, s * SB:(s + 1) * SB].any()
                         for b in range(B)]
            if all(per_b_all):
                cls[s, i] = 0
            elif not any(per_b_any):
                cls[s, i] = 1
            else:
                cls[s, i] = 2
    mixed = [(s, i) for s in range(NSB) for i in range(NTC) if cls[s, i] == 2]
    return "general", cls, mixed


def _build(mode, cls, n_mixed):
    nc = bacc.Bacc("TRN2", target_bir_lowering=False, debug=False,
                   num_devices=NCORE)
    d = {}
    for nm in ("xq", "xk", "xv"):
        d[nm] = nc.dram_tensor(nm, [D, T], BF16, kind="ExternalInput").ap()
    d["wv"] = nc.dram_tensor("wv", [D, DG], BF16, kind="ExternalInput").ap()
    # wq/wk are host-swizzled pair-major: [pair, r, c, e] so one DMA pulls
    # a single pair's weights with 2KB-contiguous per-partition rows
    for nm in ("wq", "wk"):
        d[nm] = nc.dram_tensor(nm, [NPAIR * 128 * CCH, 128], BF16,
                               kind="ExternalInput").ap()
    d["wo"] = nc.dram_tensor("wo", [DG, D], BF16, kind="ExternalInput").ap()
    d["bq"] = nc.dram_tensor("bq", [128, NPAIR], F32, kind="ExternalInput").ap()
    d["bk"] = nc.dram_tensor("bk", [128, NPAIR], F32, kind="ExternalInput").ap()
    d["bv"] = nc.dram_tensor("bv", [128, DG], BF16, kind="ExternalInput").ap()
    if n_mixed:
        d["mmask"] = nc.dram_tensor("mmask", [n_mixed, SB, TC], BF16,
                                    kind="ExternalInput").ap()
    out_d = nc.dram_tensor("out", [T, D], BF16, kind="ExternalOutput").ap()

    with tile.TileContext(nc) as tc:
        with (
            tc.tile_pool(name="persist", bufs=1) as pp,
            tc.tile_pool(name="stream", bufs=2) as sp,
            tc.tile_pool(name="small", bufs=2) as mp,
            tc.tile_pool(name="psum", bufs=2, space="PSUM") as psp,
        ):
            HV = HD + 1  # 65: V columns + ones column per head

            # ---- persistent SBUF tiles ----------------------------------
            wv_sb = pp.tile([128, CCH * DG], BF16, tag="wv")
            w_sb = {"wv": wv_sb}
            for nm in ("wq", "wk"):
                w_sb[nm] = pp.tile([128, CCH * DG], BF16, tag=nm, name=nm + "_sb")
            wo_sb = pp.tile([128, NPAIR * D], BF16, tag="wo")
            bq_sb = pp.tile([128, NPAIR], F32, tag="bq")
            bk_sb = pp.tile([128, NPAIR], F32, tag="bk")
            bvb_sb = pp.tile([128, DG], BF16, tag="bvb")
            v_sb = [pp.tile([128, 8 * HV], BF16, tag=f"v{s}", name=f"v{s}")
                    for s in range(NSB)]
            qT = [[pp.tile([128, TC], BF16, tag=f"qT{p}_{i}", name=f"qT{p}_{i}")
                   for i in range(NTC)] for p in range(NPAIR)]
            kT = [[pp.tile([128, TC], BF16, tag=f"kT{p}_{i}", name=f"kT{p}_{i}")
                   for i in range(NTC)] for p in range(NPAIR)]
            oT = [[pp.tile([128, TC], BF16, tag=f"oT{p}_{i}", name=f"oT{p}_{i}")
                   for i in range(NTC)] for p in range(NPAIR)]

            # ---- input prefetch -----------------------------------------
            # x chunk tiles keyed by tau; all big loads via gpsimd SW-DGE
            # (16 queues).  Chunk 0 + wv are split per 128-row c-chunk so
            # the first V matmul fires after ~256KB instead of ~7MB.
            x_tiles = {}

            def x_dram(nm):
                return d[nm].rearrange("(c p) t -> p c t", p=128)

            def prefetch_x(tau, fine=False):
                xs = []
                for nm in ("xv", "xq", "xk"):
                    x = sp.tile([128, CCH * PC], BF16, tag="x", bufs=6,
                                name=f"{nm}_t{tau}")
                    xs.append(x)
                x_tiles[tau] = xs
                if not fine:
                    for x, nm in zip(xs, ("xv", "xq", "xk")):
                        nc.gpsimd.dma_start(
                            out=x[:].rearrange("p (c t) -> p c t", t=PC),
                            in_=x_dram(nm)[:, :, ts(tau, PC)])

            def fine_start_loads():
                # interleave wv-chunk / xv-chunk DMAs so (wv_c0, xv_c0)
                # land first and matmuls can start immediately; the
                # qk-path loads go out in halves on the two HWDGE queues
                # (scalar/sync) + gpsimd SW-DGE concurrently.
                prefetch_x(0, fine=True)
                xv, xq, xk = x_tiles[0]

                def load_w_pair(nm, p, eng):
                    eng.dma_start(
                        out=w_sb[nm][:].rearrange(
                            "p (c e) -> p c e", e=DG)[:, :, p * 128:(p + 1) * 128],
                        in_=d[nm].rearrange(
                            "(q r c) e -> r q c e", r=128, c=CCH)[:, p, :, :])

                # gpsimd dma_start ISSUE costs ~1.1us each (SW-DGE desc
                # gen), so the startup prefix keeps gpsimd to 8 coarse
                # issues and pushes the qk-path issues onto the scalar/
                # sync HWDGE engines (~0.6us, run concurrently).  The ones
                # memsets go to the idle vector queue and the V bias comes
                # pre-broadcast from the host.
                for s in range(NSB):
                    ones_cols = v_sb[s][:].rearrange(
                        "p (h c) -> p h c", c=HV)[:, :, HD:HV]
                    nc.vector.memset(ones_cols, 1.0)
                load_w_pair("wq", 0, nc.scalar)
                load_w_pair("wk", 0, nc.sync)
                for c in range(CCH):
                    nc.gpsimd.dma_start(
                        out=wv_sb[:, ts(c, DG)],
                        in_=d["wv"].rearrange("(c p) e -> p c e", p=128)[:, c, :])
                    nc.gpsimd.dma_start(
                        out=xv[:, ts(c, PC)],
                        in_=x_dram("xv")[:, c, ts(0, PC)])
                for h in range(2):
                    nc.gpsimd.dma_start(
                        out=xq[:].rearrange("p (c t) -> p c t", t=PC)[:, 4 * h:4 * h + 4, :],
                        in_=x_dram("xq")[:, 4 * h:4 * h + 4, ts(0, PC)])
                    nc.gpsimd.dma_start(
                        out=xk[:].rearrange("p (c t) -> p c t", t=PC)[:, 4 * h:4 * h + 4, :],
                        in_=x_dram("xk")[:, 4 * h:4 * h + 4, ts(0, PC)])
                nc.sync.dma_start(out=bvb_sb[:], in_=d["bv"][:])
                for p_, eng in ((1, nc.scalar), (2, nc.sync), (3, nc.scalar)):
                    load_w_pair("wq", p_, eng)
                    load_w_pair("wk", p_, eng)
                nc.sync.dma_start(out=bq_sb[:], in_=d["bq"][:])
                nc.sync.dma_start(out=bk_sb[:], in_=d["bk"][:])
                nc.gpsimd.dma_start(
                    out=wo_sb[:].rearrange("p (c e) -> p c e", e=D),
                    in_=d["wo"].rearrange("(c p) e -> p c e", p=128))

            # ---- V projection (per 128-token group) ---------------------
            def emit_v_group(tau, u):
                x = x_tiles[tau][0]
                sigma = tau * (PC // SB) + u
                ps = psp.tile([128, TC], F32, tag="b512", bufs=2)
                for c in range(CCH):
                    nc.tensor.matmul(
                        ps[:],
                        x[:, c * PC + u * SB:c * PC + (u + 1) * SB],
                        wv_sb[:, ts(c, DG)],
                        start=(c == 0), stop=(c == CCH - 1))
                vdst = v_sb[sigma][:].rearrange("p (h c) -> p h c", c=HV)[:, :, 0:HD]
                vsrc = ps[:].rearrange("p (h c) -> p h c", c=HD)
                bvv = bvb_sb[:].rearrange("p (h c) -> p h c", c=HD)
                nc.vector.tensor_add(vdst, vsrc, bvv)

            # ---- Q/K projections (per (pair, q-or-k) psum group) --------
            def emit_qk_one(tau, p, nm):
                xx = x_tiles[tau][1 if nm == "q" else 2]
                dst, bias = (qT, bq_sb) if nm == "q" else (kT, bk_sb)
                ps = psp.tile([128, TC], F32, tag="b512", bufs=2)
                for c in range(CCH):
                    nc.tensor.matmul(
                        ps[:],
                        w_sb["w" + nm][:, c * DG + p * 128:c * DG + (p + 1) * 128],
                        xx[:, ts(c, PC)],
                        start=(c == 0), stop=(c == CCH - 1))
                nc.vector.tensor_scalar(
                    out=dst[p][tau][:], in0=ps[:],
                    scalar1=bias[:, p:p + 1], scalar2=None,
                    op0=mybir.AluOpType.add)

            scale = 1.0 / math.sqrt(HD)

            def build_unit(i, p):
                """Returns (st_fns, pv_fns, epi_fn) for attention unit (i,p)."""
                if mode == "causal":
                    blocks = []
                    for s_blk in range(4 * i + 4):
                        j = s_blk - 4 * i
                        if j < 0:
                            blocks.append((s_blk, i * TC, TC, False))
                        else:
                            s0 = SB * s_blk
                            toff = s0 if j < 3 else s0 - SB
                            blocks.append((s_blk, toff, TC * (i + 1) - toff, True))
                else:
                    blocks = [(s_blk, i * TC, TC, False)
                              for s_blk in range(NSB) if cls[s_blk, i] != 0]
                state = {"p2": {}, "ot": None}

                def make_st(bi):
                    s_blk, toff, n, diag = blocks[bi]

                    def fn():
                        s0 = SB * s_blk
                        sc, lo = s_blk // 4, SB * (s_blk % 4)
                        tl = toff - i * TC
                        st2 = psp.tile([128, 2 * TC], F32, tag="stAB", bufs=2,
                                       name="st2")
                        nc.tensor.matmul(
                            st2[:, 0:n], kT[p][sc][0:HD, lo:lo + SB],
                            qT[p][i][0:HD, tl:tl + n],
                            start=True, stop=True, tile_position=(0, 0))
                        nc.tensor.matmul(
                            st2[:, TC:TC + n], kT[p][sc][HD:128, lo:lo + SB],
                            qT[p][i][HD:128, tl:tl + n],
                            start=True, stop=True, tile_position=(64, 0))
                        p2 = sp.tile([128, 2 * TC], BF16, tag="pAB", bufs=6,
                                     name="p2")
                        if n == TC:
                            nc.scalar.activation(p2[:], st2[:], AF.Exp, scale=scale)
                        else:
                            st3 = st2[:].rearrange("p (b c) -> p b c", b=2)[:, :, 0:n]
                            p3 = p2[:].rearrange("p (b c) -> p b c", b=2)[:, :, 0:n]
                            nc.scalar.activation(p3, st3, AF.Exp, scale=scale)
                        if mode == "causal" and diag:
                            w_ = s0 + SB - toff
                            p3w = p2[:].rearrange("p (b c) -> p b c", b=2)[:, :, 0:w_]
                            nc.gpsimd.affine_select(
                                out=p3w, in_=p3w,
                                compare_op=mybir.AluOpType.is_ge,
                                fill=0.0, base=toff - s0,
                                pattern=[[0, 2], [1, w_]], channel_multiplier=-1)
                        elif mode == "general" and cls[s_blk, i] == 2:
                            mmt = sp.tile([SB, TC], BF16, tag="mmask", name="mmt")
                            nc.sync.dma_start(out=mmt[:],
                                              in_=d["mmask"][mixed_idx[(s_blk, i)]])
                            for off in (0, TC):
                                nc.vector.tensor_mul(p2[:, off:off + n],
                                                     p2[:, off:off + n], mmt[:, 0:n])
                        state["p2"][bi] = p2
                    return fn

                def make_pv(bi):
                    s_blk, toff, n, diag = blocks[bi]

                    def fn():
                        if state["ot"] is None:
                            state["ot"] = (
                                psp.tile([HV, TC], F32, tag="ot", bufs=2, name="otA"),
                                psp.tile([HV, TC], F32, tag="ot", bufs=2, name="otB"))
                        otA, otB = state["ot"]
                        p2 = state["p2"].pop(bi)
                        tl = toff - i * TC
                        vv = v_sb[s_blk][:].rearrange("p (h c) -> p h c", c=HV)
                        first, last = bi == 0, bi == len(blocks) - 1
                        nc.tensor.matmul(otA[:, tl:tl + n], vv[:, 2 * p, :],
                                         p2[:, 0:n], start=first, stop=last)
                        nc.tensor.matmul(otB[:, tl:tl + n], vv[:, 2 * p + 1, :],
                                         p2[:, TC:TC + n], start=first, stop=last)
                    return fn

                def epi():
                    otA, otB = state["ot"]
                    # head B first: its extra SBUF->SBUF hop (stg DMA into
                    # partitions 64..127) is the long pole before the
                    # out-projection can consume oT.
                    for hh, ot_ps in ((1, otB), (0, otA)):
                        den = mp.tile([HV, TC], F32, tag="den", name="den")
                        nc.vector.tensor_copy(den[HD:HV, :], ot_ps[HD:HV, :])
                        r1 = mp.tile([1, TC], F32, tag="r1", name="r1")
                        nc.sync.dma_start(out=r1[:], in_=den[HD:HV, :])
                        rbd = mp.tile([HD, TC], F32, tag="rbd", name="rbd")
                        nc.gpsimd.partition_broadcast(rbd[:], r1[:])
                        rb = mp.tile([HD, TC], F32, tag="rb", name="rb")
                        nc.vector.reciprocal_approx_fast(out=rb[:], in_=rbd[:])
                        if hh == 0:
                            nc.vector.tensor_mul(oT[p][i][0:HD, :], ot_ps[0:HD, :],
                                                 rb[:])
                        else:
                            stg = mp.tile([HD, TC], BF16, tag="stg", name="stg")
                            nc.vector.tensor_mul(stg[:], ot_ps[0:HD, :], rb[:])
                            nc.sync.dma_start(out=oT[p][i][HD:128, :], in_=stg[:])

                n = len(blocks)
                return [make_st(b) for b in range(n)], [make_pv(b) for b in range(n)], epi

            # ---- out-projection (per (chunk, tt) piece: both e-halves) --
            def emit_outproj_tt(i, tt):
                ob = sp.tile([128, D], BF16, tag="ob", bufs=3, name="ob")
                for e in range(2):
                    ps = psp.tile([128, TC], F32, tag="b512", bufs=2, name="ops")
                    for p in range(NPAIR):
                        nc.tensor.matmul(
                            ps[:], oT[p][i][:, ts(tt - 4 * i, 128)],
                            wo_sb[:, p * D + e * TC:p * D + (e + 1) * TC],
                            start=(p == 0), stop=(p == NPAIR - 1))
                    nc.vector.tensor_copy(ob[:, ts(e, TC)], ps[:])
                    # per-half DMA on alternating queues: e=0 overlaps the
                    # e=1 matmuls, and the final store drains ~2x faster
                    eng = nc.sync if e == 0 else nc.gpsimd
                    eng.dma_start(out=out_d[ts(tt, 128), ts(e, TC)],
                                  in_=ob[:, ts(e, TC)])

            # ---- schedule ------------------------------------------------
            # LAG-pipelined attention emission (PVs lag STs by LAG blocks,
            # carried across unit boundaries) with a per-chunk "pieces"
            # queue of PE filler drained between blocks inside each unit.
            LAG = 1
            carry = []

            def emit_unit(st_fns, pv_fns, epi, pieces, quota):
                prev = carry[:]
                carry.clear()
                nb = len(st_fns)
                done = 0
                for b in range(nb):
                    st_fns[b]()
                    if b < len(prev):
                        prev[b]()
                    if b >= LAG:
                        pv_fns[b - LAG]()
                    want = min(quota * (b + 1) // nb, len(pieces))
                    while done < want:
                        pieces.pop(0)()
                        done -= -1
                for fn in prev[nb:]:
                    fn()
                carry.extend(pv_fns[max(nb - LAG, 0):])
                carry.append(epi)

            fine_start_loads()
            for u in range(PC // SB):
                emit_v_group(0, u)
            emit_qk_one(0, 0, "q")
            emit_qk_one(0, 0, "k")

            for i in range(NTC):
                pieces = []
                t2 = i + 1
                if t2 < NTC:
                    pieces.append(lambda t2=t2: prefetch_x(t2))
                    for u in range(PC // SB):
                        pieces.append(lambda t2=t2, u=u: emit_v_group(t2, u))
                    for p_ in range(NPAIR):
                        for nm in ("q", "k"):
                            pieces.append(
                                lambda t2=t2, p_=p_, nm=nm: emit_qk_one(t2, p_, nm))
                else:
                    for j in range(NTC - 1):
                        for tt in range(4 * j, 4 * j + 4):
                            pieces.append(lambda j=j, tt=tt: emit_outproj_tt(j, tt))
                for p in range(NPAIR):
                    if i == 0 and p + 1 < NPAIR:
                        # chunk 0: q/k for pair p+1 must precede unit (0,p+1)
                        emit_qk_one(0, p + 1, "q")
                        emit_qk_one(0, p + 1, "k")
                    st_fns, pv_fns, epi = build_unit(i, p)
                    quota = -(-len(pieces) // (NPAIR - p))
                    emit_unit(st_fns, pv_fns, epi, pieces, quota)
                for fn in pieces:
                    fn()
            # tail: overlap the last unit's epilogue chain with partial
            # out-projection accumulation (pairs 0..2 need no epilogue of
            # the final pair).  tt=12 uses the b512 pool pair, tt=13 the
            # just-freed stAB-sized psum (two 512-col halves).
            *pvs, epi_last = carry
            for fn in pvs:
                fn()
            i3 = NTC - 1
            ps12 = [psp.tile([128, TC], F32, tag="b512", bufs=2, name="ops")
                    for _ in range(2)]
            ps13 = psp.tile([128, 2 * TC], F32, tag="stAB", bufs=2, name="ops13")
            ps14 = psp.tile([128, 2 * TC], F32, tag="stAB", bufs=2, name="ops14")
            part = {(12, 0): ps12[0][:], (12, 1): ps12[1][:],
                    (13, 0): ps13[:, 0:TC], (13, 1): ps13[:, TC:2 * TC],
                    (14, 0): ps14[:, 0:TC], (14, 1): ps14[:, TC:2 * TC]}
            for (tt, e), ps in part.items():
                for p in range(NPAIR - 1):
                    nc.tensor.matmul(
                        ps, oT[p][i3][:, ts(tt - 4 * i3, 128)],
                        wo_sb[:, p * D + e * TC:p * D + (e + 1) * TC],
                        start=(p == 0), stop=False)
            epi_last()
            for tt in (12, 13, 14):
                ob = sp.tile([128, D], BF16, tag="ob", bufs=3, name="ob")
                for e in range(2):
                    ps = part[(tt, e)]
                    nc.tensor.matmul(
                        ps, oT[NPAIR - 1][i3][:, ts(tt - 4 * i3, 128)],
                        wo_sb[:, (NPAIR - 1) * D + e * TC:(NPAIR - 1) * D + (e + 1) * TC],
                        start=False, stop=True)
                    nc.vector.tensor_copy(ob[:, ts(e, TC)], ps)
                    eng = nc.sync if e == 0 else nc.gpsimd
                    eng.dma_start(out=out_d[ts(tt, 128), ts(e, TC)],
                                  in_=ob[:, ts(e, TC)])
            emit_outproj_tt(i3, 15)

    nc.compile()
    return nc


def kernel(**inputs):
    query = np.asarray(inputs["query"], np.float32)
    key = np.asarray(inputs["key"], np.float32)
    value = np.asarray(inputs["value"], np.float32)
    mask = np.asarray(inputs["mask"], bool)
    Wq, bq = np.asarray(inputs["Wq"], np.float32), np.asarray(inputs["bq"], np.float32)
    Wk, bk = np.asarray(inputs["Wk"], np.float32), np.asarray(inputs["bk"], np.float32)
    Wv, bv = np.asarray(inputs["Wv"], np.float32), np.asarray(inputs["bv"], np.float32)
    Wo, bo = np.asarray(inputs["Wo"], np.float32), np.asarray(inputs["bo"], np.float32)

    mode, cls, mixed = _classify_blocks(mask)
    global mixed_idx
    if mode == "general":
        mixed_idx = {blk: n for n, blk in enumerate(mixed)}
        n_mixed = len(mixed)
    else:
        mixed_idx, n_mixed = {}, 0

    key_sig = (mode, tuple(cls.ravel()) if cls is not None else None)
    if key_sig not in _cache:
        _cache[key_sig] = _build(mode, cls, n_mixed)
    nc = _cache[key_sig]

    in_maps = []
    xT = {}
    for b in range(B):
        xT[("xq", b)] = np.ascontiguousarray(query[b].T).astype(ml_dtypes.bfloat16)
        xT[("xk", b)] = np.ascontiguousarray(key[b].T).astype(ml_dtypes.bfloat16)
        xT[("xv", b)] = np.ascontiguousarray(value[b].T).astype(ml_dtypes.bfloat16)
    for core in range(NCORE):
        b, g = core // 2, core % 2
        sl = slice(g * DG, (g + 1) * DG)
        im = {
            "xq": xT[("xq", b)], "xk": xT[("xk", b)], "xv": xT[("xv", b)],
            # pair-major swizzle [pair, r, c, e] flattened to 2D
            "wq": np.ascontiguousarray(
                Wq[sl, :].T.reshape(CCH, 128, NPAIR, 128)
                .transpose(2, 1, 0, 3).reshape(NPAIR * 128 * CCH, 128)
            ).astype(ml_dtypes.bfloat16),
            "wk": np.ascontiguousarray(
                Wk[sl, :].T.reshape(CCH, 128, NPAIR, 128)
                .transpose(2, 1, 0, 3).reshape(NPAIR * 128 * CCH, 128)
            ).astype(ml_dtypes.bfloat16),
            "wv": np.ascontiguousarray(Wv[sl, :].T).astype(ml_dtypes.bfloat16),
            "wo": np.ascontiguousarray(Wo[:, sl].T).astype(ml_dtypes.bfloat16),
            "bq": np.ascontiguousarray(bq[sl].reshape(NPAIR, 128).T),
            "bk": np.ascontiguousarray(bk[sl].reshape(NPAIR, 128).T),
            "bv": np.ascontiguousarray(
                np.broadcast_to(bv[sl][None, :], (128, DG))).astype(ml_dtypes.bfloat16),
        }
        if n_mixed:
            mm = np.empty((n_mixed, SB, TC), ml_dtypes.bfloat16)
            for n, (s_blk, i) in enumerate(mixed):
                blk = mask[b, i * TC:(i + 1) * TC, s_blk * SB:(s_blk + 1) * SB]
                mm[n] = (~blk.T).astype(np.float32)
            im["mmask"] = mm
        in_maps.append(im)

    r = run_bass_kernel_spmd(nc, in_maps, core_ids=list(range(NCORE)))
    last_result["exec_time_ns"] = r.exec_time_ns
    out = np.empty((B, T, D), np.float32)
    for b in range(B):
        out[b] = (r.results[2 * b]["out"].astype(np.float32)
                  + r.results[2 * b + 1]["out"].astype(np.float32))
    out += bo[None, None, :]
    return out
